# revision 1
# baseline (speedup 1.0000x reference)
"""Trainium2 Bass kernel for a 6-layer GPT forward pass (nn_GPT_21019569946962).

Sharding: sequence-parallel. 8 cores = 2 batches x 4 chunks of 256 tokens.
Per layer: each core computes LN1/QKV on its 256 tokens, AllGathers K and V
within its 4-core (per-batch) replica group, runs causal attention over the
gathered keys with additive -30000 masks, then WO/LN2/MLP locally.  The
final LN output is AllGathered across all 8 cores and the LM head is
vocab-sharded (each core computes a 6288-column slice of the vocabulary for
all 2048 tokens).

Activations flow feature-major [D, tokens]; weights stream from HBM in
bf16; the residual stream and softmax/LN statistics stay fp32.
"""

import os
import sys

sys.path.insert(0, "/opt/trn_rl_repo")

import numpy as np
import ml_dtypes

import concourse.bass as bass
import concourse.tile as tile
import concourse.mybir as mybir
from concourse import bacc
from concourse import bass_utils
from concourse.masks import make_identity

BF16 = mybir.dt.bfloat16
F32 = mybir.dt.float32
AF = mybir.ActivationFunctionType
ALU = mybir.AluOpType

N_CORES = 8
NL = 6          # layers
D = 768
DT = 6          # d-tiles of 128
H = 12          # heads
HD = 64         # head dim
DFF = 3072
DFT = 24        # dff tiles of 128
VOC = 50304
VS = VOC // N_CORES  # 6288 vocab shard per core
B, L = 2, 1024
TOK = 256       # tokens per core
NEG = -30000.0
EPS = 1e-6


class GptKernel:
    def __init__(self, reps=1):
        self.reps = reps
        self.nc = self._build()

    # -------------------------------------------------------------- build
    def _build(self):
        nc = bacc.Bacc("TRN2", target_bir_lowering=False, debug=False,
                       enable_asserts=True, num_devices=N_CORES)
        self.nc = nc

        def din(name, shape, dt):
            return nc.dram_tensor(name, shape, dt, kind="ExternalInput").ap()

        self.x0 = din("x0", [D, TOK], F32)
        self.wq = din("wq", [NL, D, D], BF16)
        self.wk = din("wk", [NL, D, D], BF16)
        self.wv = din("wv", [NL, D, D], BF16)
        self.wo = din("wo", [NL, D, D], BF16)
        self.w1 = din("w1", [NL, D, DFF], BF16)
        self.w2 = din("w2", [NL, DFF, D], BF16)
        self.w1b = din("w1b", [NL, DFF], F32)
        self.w2b = din("w2b", [NL, D], F32)
        self.ln1s = din("ln1s", [NL, D], F32)
        self.ln1b = din("ln1b", [NL, D], F32)
        self.ln2s = din("ln2s", [NL, D], F32)
        self.ln2b = din("ln2b", [NL, D], F32)
        self.lnfs = din("lnfs", [D], F32)
        self.lnfb = din("lnfb", [D], F32)
        self.headw = din("headw", [D, VS], BF16)
        self.amask = din("amask", [8, 128, TOK], F32)
        self.out = nc.dram_tensor("out", [N_CORES * TOK, VS], F32,
                                  kind="ExternalOutput").ap()

        with tile.TileContext(nc) as tc:
            self.tc = tc
            with (
                tc.tile_pool(name="const", bufs=1) as cp,
                tc.tile_pool(name="persist", bufs=1) as pp,
                tc.tile_pool(name="psum", bufs=1, space="PSUM") as psum,
                tc.tile_pool(name="dram", bufs=1, space="DRAM") as dram,
                tc.tile_pool(name="work", bufs=1) as wp,
            ):
                self.psum, self.dram, self.wp = psum, dram, wp
                self.ones_r = cp.tile([1, 128], F32)
                nc.vector.memset(self.ones_r[:], 1.0)
                self.ones_c = cp.tile([128, 1], BF16)
                nc.vector.memset(self.ones_c[:], 1.0)
                self.ident = cp.tile([128, 128], BF16)
                make_identity(nc, self.ident[:])

                self.mask_sb = pp.tile([128, 8, TOK], F32)
                nc.sync.dma_start(self.mask_sb[:],
                                  self.amask.rearrange("k p t -> p k t"))
                self.xres = pp.tile([128, DT, TOK], F32)

                for rep in range(self.reps):
                    nc.sync.dma_start(
                        self.xres[:],
                        self.x0.rearrange("(dt p) t -> p dt t", p=128))
                    for l in range(NL):
                        self._layer(l, rep)
                    self._lm_head(rep)
        nc.compile()
        return nc

    # ------------------------------------------------------------ pieces
    def _layernorm(self, xres, g, b, name):
        nc, wp, psum = self.nc, self.wp, self.psum
        stat_s = psum.tile([1, TOK], F32, tag="small", bufs=2, name=f"ss_{name}")
        stat_q = psum.tile([1, TOK], F32, tag="small", bufs=2, name=f"sq_{name}")
        xbs = []
        for k in range(DT):
            xb = wp.tile([128, TOK], BF16, tag="xb", bufs=8, name=f"xb{k}_{name}")
            nc.vector.tensor_copy(xb[:], xres[:, k, :])
            xbs.append(xb)
        for k in range(DT):
            nc.tensor.matmul(stat_s[:], self.ones_c[:], xbs[k][:],
                             start=(k == 0), stop=(k == DT - 1))
        xqs = []
        for k in range(DT):
            xq = wp.tile([128, TOK], BF16, tag="xq", bufs=8, name=f"xq{k}_{name}")
            nc.vector.tensor_mul(xq[:], xbs[k][:], xbs[k][:])
            xqs.append(xq)
        for k in range(DT):
            nc.tensor.matmul(stat_q[:], self.ones_c[:], xqs[k][:],
                             start=(k == 0), stop=(k == DT - 1))
        mu = wp.tile([1, TOK], F32, tag="lnsc", bufs=6, name=f"mu_{name}")
        nc.vector.tensor_scalar_mul(mu[:], stat_s[:], 1.0 / D)
        msq = wp.tile([1, TOK], F32, tag="lnsc", bufs=6, name=f"msq_{name}")
        nc.vector.tensor_scalar_mul(msq[:], stat_q[:], 1.0 / D)
        mu2 = wp.tile([1, TOK], F32, tag="lnsc", bufs=6, name=f"mu2_{name}")
        nc.vector.tensor_mul(mu2[:], mu[:], mu[:])
        var = wp.tile([1, TOK], F32, tag="lnsc", bufs=6, name=f"var_{name}")
        nc.vector.tensor_sub(var[:], msq[:], mu2[:])
        vare = wp.tile([1, TOK], F32, tag="lnsc", bufs=6, name=f"ve_{name}")
        nc.vector.tensor_scalar_add(vare[:], var[:], EPS)
        sd = wp.tile([1, TOK], F32, tag="lnsc", bufs=6, name=f"sd_{name}")
        nc.scalar.activation(sd[:], vare[:], AF.Sqrt, bias=0.0, scale=1.0)
        rstd = wp.tile([1, TOK], F32, tag="lnsc", bufs=6, name=f"rstd_{name}")
        nc.vector.reciprocal(rstd[:], sd[:])

        bc_m = psum.tile([128, TOK], F32, tag="s", bufs=4, name=f"bcm_{name}")
        nc.tensor.matmul(bc_m[:], self.ones_r[:], mu[:], start=True, stop=True)
        bc_r = psum.tile([128, TOK], F32, tag="s", bufs=4, name=f"bcr_{name}")
        nc.tensor.matmul(bc_r[:], self.ones_r[:], rstd[:], start=True, stop=True)

        ln = wp.tile([128, DT, TOK], BF16, tag="lnx", bufs=2, name=f"ln_{name}")
        for k in range(DT):
            u = wp.tile([128, TOK], F32, tag="lnu", bufs=3, name=f"u{k}_{name}")
            nc.vector.tensor_sub(u[:], xres[:, k, :], bc_m[:])
            v = wp.tile([128, TOK], F32, tag="lnv", bufs=3, name=f"v{k}_{name}")
            nc.vector.tensor_mul(v[:], u[:], bc_r[:])
            nc.scalar.activation(ln[:, k, :], v[:], AF.Identity,
                                 bias=b[:, k:k + 1], scale=g[:, k:k + 1])
        return ln

    def _scales(self, src, l, name):
        nc, wp = self.nc, self.wp
        t = wp.tile([128, DT], F32, tag="sc", bufs=14, name=name)
        nc.sync.dma_start(t[:], src[l].rearrange("(t p) -> p t", p=128))
        return t

    def _layer(self, l, rep):
        nc, wp, psum, dram = self.nc, self.wp, self.psum, self.dram
        nm = f"r{rep}l{l}"
        g1 = self._scales(self.ln1s, l, f"g1_{nm}")
        b1 = self._scales(self.ln1b, l, f"b1_{nm}")
        g2 = self._scales(self.ln2s, l, f"g2_{nm}")
        b2 = self._scales(self.ln2b, l, f"b2_{nm}")
        w2b_s = self._scales(self.w2b, l, f"w2b_{nm}")
        w1b_s = wp.tile([128, DFT], F32, tag="sc24", bufs=2, name=f"w1b_{nm}")
        nc.sync.dma_start(w1b_s[:], self.w1b[l].rearrange("(t p) -> p t", p=128))

        ln1 = self._layernorm(self.xres, g1, b1, f"ln1_{nm}")

        def proj_hm(wdram, outname):
            wts = []
            for kk in range(DT):
                wt = wp.tile([128, D], BF16, tag="wproj", bufs=8,
                             name=f"w_{outname}{kk}_{nm}")
                nc.sync.dma_start(wt[:], wdram[l, kk * 128:(kk + 1) * 128, :])
                wts.append(wt)
            hm = wp.tile([64, H, TOK], BF16, tag=outname, bufs=1,
                         name=f"{outname}_{nm}")
            for h in range(H):
                ps = psum.tile([64, TOK], F32, tag="s", bufs=4,
                               name=f"p_{outname}{h}_{nm}")
                for kk in range(DT):
                    nc.tensor.matmul(ps[:], wts[kk][:, h * 64:(h + 1) * 64],
                                     ln1[:, kk, :],
                                     start=(kk == 0), stop=(kk == DT - 1))
                nc.vector.tensor_copy(hm[:, h, :], ps[:])
            return hm

        # ---- K projection + AllGather
        k_hm = proj_hm(self.wk, "khm")
        kin = dram.tile([H, 64, TOK], BF16, tag="kin", bufs=2, name=f"kin_{nm}")
        for h in range(H):
            nc.sync.dma_start(kin[h], k_hm[:, h, :])
        kout = dram.tile([4, H, 64, TOK], BF16, tag="kout", bufs=2,
                         name=f"kout_{nm}")
        nc.gpsimd.collective_compute(
            "AllGather", ALU.bypass, ins=[kin.opt()], outs=[kout.opt()],
            replica_groups=[[0, 1, 2, 3], [4, 5, 6, 7]])
        kg = wp.tile([64, 4, H, TOK], BF16, tag="kg", bufs=1, name=f"kg_{nm}")
        for r in range(4):
            nc.sync.dma_start(kg[:, r], kout[r].rearrange("h d t -> d h t"))

        # ---- V projection (d-major) -> token-major (+ones col) + AllGather
        vts = []
        for kk in range(DT):
            wt = wp.tile([128, D], BF16, tag="wproj", bufs=8,
                         name=f"w_v{kk}_{nm}")
            nc.sync.dma_start(wt[:], self.wv[l, kk * 128:(kk + 1) * 128, :])
            vts.append(wt)
        vdm = wp.tile([128, DT, TOK], BF16, tag="vdm", bufs=1, name=f"vdm_{nm}")
        for m in range(DT):
            ps = psum.tile([128, TOK], F32, tag="s", bufs=4, name=f"pv{m}_{nm}")
            for kk in range(DT):
                nc.tensor.matmul(ps[:], vts[kk][:, m * 128:(m + 1) * 128],
                                 ln1[:, kk, :],
                                 start=(kk == 0), stop=(kk == DT - 1))
            nc.vector.tensor_copy(vdm[:, m, :], ps[:])
        vtok = wp.tile([128, 2, H * 65], BF16, tag="vtok", bufs=1,
                       name=f"vtok_{nm}")
        nc.gpsimd.memset(vtok[:], 1.0)
        for dt_ in range(DT):
            for h2 in range(2):
                tp = psum.tile([128, 128], BF16, tag="s", bufs=4,
                               name=f"tp{dt_}_{h2}_{nm}")
                nc.tensor.transpose(tp[:], vdm[:, dt_, h2 * 128:(h2 + 1) * 128],
                                    self.ident[:])
                nc.vector.tensor_copy(
                    vtok[:, h2, 65 * (2 * dt_):65 * (2 * dt_) + 64],
                    tp[:, 0:64])
                nc.vector.tensor_copy(
                    vtok[:, h2, 65 * (2 * dt_ + 1):65 * (2 * dt_ + 1) + 64],
                    tp[:, 64:128])
        vin = dram.tile([2, 128, H * 65], BF16, tag="vin", bufs=2,
                        name=f"vin_{nm}")
        for h2 in range(2):
            nc.sync.dma_start(vin[h2], vtok[:, h2, :])
        vout = dram.tile([4, 2, 128, H * 65], BF16, tag="vout", bufs=2,
                         name=f"vout_{nm}")
        nc.gpsimd.collective_compute(
            "AllGather", ALU.bypass, ins=[vin.opt()], outs=[vout.opt()],
            replica_groups=[[0, 1, 2, 3], [4, 5, 6, 7]])
        vg = wp.tile([128, 8, H * 65], BF16, tag="vg", bufs=1, name=f"vg_{nm}")
        for r in range(4):
            for h2 in range(2):
                nc.sync.dma_start(vg[:, r * 2 + h2, :], vout[r, h2])

        # ---- Q projection
        q_hm = proj_hm(self.wq, "qhm")

        # ---- attention
        at = wp.tile([64, H, TOK], BF16, tag="at", bufs=1, name=f"at_{nm}")
        for h in range(H):
            oaug = psum.tile([65, TOK], F32, tag="small", bufs=2,
                             name=f"oaug{h}_{nm}")
            for kt in range(8):
                r, h2 = kt // 2, kt % 2
                s_ps = psum.tile([128, TOK], F32, tag="s", bufs=4,
                                 name=f"s{h}_{kt}_{nm}")
                nc.tensor.matmul(
                    s_ps[:], self_kg_slice(kg, r, h, h2),
                    q_hm[:, h, :], start=True, stop=True)
                sm = wp.tile([128, TOK], F32, tag="sm", bufs=4,
                             name=f"sm{h}_{kt}_{nm}")
                nc.vector.tensor_add(sm[:], s_ps[:], self.mask_sb[:, kt, :])
                p = wp.tile([128, TOK], BF16, tag="pexp", bufs=6,
                            name=f"pe{h}_{kt}_{nm}")
                nc.scalar.activation(p[:], sm[:], AF.Exp, bias=0.0, scale=0.125)
                nc.tensor.matmul(oaug[:], vg[:, kt, 65 * h:65 * h + 65], p[:],
                                 start=(kt == 0), stop=(kt == 7))
            dnr = wp.tile([1, TOK], F32, tag="dnr", bufs=2, name=f"dnr{h}_{nm}")
            nc.vector.reciprocal(dnr[:], oaug[64:65, :])
            bc = psum.tile([64, TOK], F32, tag="s", bufs=4, name=f"bc{h}_{nm}")
            nc.tensor.matmul(bc[:], self.ones_r[:, 0:64], dnr[:],
                             start=True, stop=True)
            ou = wp.tile([64, TOK], F32, tag="ou", bufs=2, name=f"ou{h}_{nm}")
            nc.vector.tensor_copy(ou[:], oaug[0:64, :])
            nc.vector.tensor_mul(at[:, h, :], ou[:], bc[:])

        # ---- WO + residual
        wo_hm = wp.tile([64, H, D], BF16, tag="wohm", bufs=1, name=f"wo_{nm}")
        for h in range(H):
            nc.sync.dma_start(wo_hm[:, h, :], self.wo[l, h * 64:(h + 1) * 64, :])
        for m in range(DT):
            ps = psum.tile([128, TOK], F32, tag="dense", bufs=2,
                           name=f"pwo{m}_{nm}")
            for h in range(H):
                nc.tensor.matmul(ps[:], wo_hm[:, h, m * 128:(m + 1) * 128],
                                 at[:, h, :], start=(h == 0), stop=(h == H - 1))
            nc.vector.tensor_add(self.xres[:, m, :], self.xres[:, m, :], ps[:])

        # ---- LN2 + MLP
        ln2 = self._layernorm(self.xres, g2, b2, f"ln2_{nm}")
        h1 = wp.tile([128, DFT, TOK], BF16, tag="h1", bufs=1, name=f"h1_{nm}")
        for half in range(2):
            w1ts = []
            for kk in range(DT):
                wt = wp.tile([128, DFF // 2], BF16, tag="w1", bufs=7,
                             name=f"w1_{kk}_{half}_{nm}")
                nc.sync.dma_start(
                    wt[:], self.w1[l, kk * 128:(kk + 1) * 128,
                                   half * (DFF // 2):(half + 1) * (DFF // 2)])
                w1ts.append(wt)
            for mi in range(DFT // 2):
                m = half * (DFT // 2) + mi
                ps = psum.tile([128, TOK], F32, tag="dense", bufs=2,
                               name=f"ph1_{m}_{nm}")
                for kk in range(DT):
                    nc.tensor.matmul(ps[:], w1ts[kk][:, mi * 128:(mi + 1) * 128],
                                     ln2[:, kk, :],
                                     start=(kk == 0), stop=(kk == DT - 1))
                nc.scalar.activation(h1[:, m, :], ps[:], AF.Gelu_apprx_tanh,
                                     bias=w1b_s[:, m:m + 1], scale=1.0)
        for mb in range(3):
            w2ts = []
            for kk in range(DFT):
                wt = wp.tile([128, D], BF16, tag="w2", bufs=5,
                             name=f"w2_{mb}_{kk}_{nm}")
                nc.sync.dma_start(wt[:], self.w2[l, kk * 128:(kk + 1) * 128, :])
                w2ts.append(wt)
            for mi in range(2):
                m = mb * 2 + mi
                ps = psum.tile([128, TOK], F32, tag="dense", bufs=2,
                               name=f"pw2_{m}_{nm}")
                for kk in range(DFT):
                    nc.tensor.matmul(ps[:], w2ts[kk][:, m * 128:(m + 1) * 128],
                                     h1[:, kk, :],
                                     start=(kk == 0), stop=(kk == DFT - 1))
                mo = wp.tile([128, TOK], F32, tag="mo", bufs=3,
                             name=f"mo{m}_{nm}")
                nc.scalar.activation(mo[:], ps[:], AF.Identity,
                                     bias=w2b_s[:, m:m + 1], scale=1.0)
                nc.vector.tensor_add(self.xres[:, m, :], self.xres[:, m, :],
                                     mo[:])

    def _lm_head(self, rep):
        nc, wp, psum, dram = self.nc, self.wp, self.psum, self.dram
        nm = f"r{rep}f"
        gf = self._scales_1d(self.lnfs, f"gf_{nm}")
        bf = self._scales_1d(self.lnfb, f"bf_{nm}")
        lnf = self._layernorm(self.xres, gf, bf, f"lnf_{nm}")
        fin = dram.tile([DT, 128, TOK], BF16, tag="fin", bufs=1, name=f"fin_{nm}")
        for k in range(DT):
            nc.sync.dma_start(fin[k], lnf[:, k, :])
        fout = dram.tile([N_CORES, DT, 128, TOK], BF16, tag="fout", bufs=1,
                         name=f"fout_{nm}")
        nc.gpsimd.collective_compute(
            "AllGather", ALU.bypass, ins=[fin.opt()], outs=[fout.opt()],
            replica_groups=[[0, 1, 2, 3, 4, 5, 6, 7]])
        fg = wp.tile([128, N_CORES, DT, TOK], BF16, tag="kg", bufs=1,
                     name=f"fg_{nm}")
        for r in range(N_CORES):
            nc.sync.dma_start(fg[:, r], fout[r].rearrange("k p t -> p k t"))

        vblocks = [(i * 1536, 1536) for i in range(4)] + [(6144, 144)]
        for vb0, vbn in vblocks:
            hts = []
            for kk in range(DT):
                ht = wp.tile([128, DFF // 2], BF16, tag="w1", bufs=7,
                             name=f"hw{vb0}_{kk}_{nm}")
                nc.sync.dma_start(
                    ht[:, :vbn],
                    self.headw[kk * 128:(kk + 1) * 128, vb0:vb0 + vbn])
                hts.append(ht)
            nvc = (vbn + 511) // 512
            for tt in range(16):
                r, h2 = tt // 2, tt % 2
                for vc in range(nvc):
                    n = min(512, vbn - vc * 512)
                    ps = psum.tile([128, 512], F32, tag="dense", bufs=2,
                                   name=f"po{vb0}_{tt}_{vc}_{nm}")
                    for kk in range(DT):
                        nc.tensor.matmul(
                            ps[:, :n],
                            fg[:, r, kk, h2 * 128:(h2 + 1) * 128],
                            hts[kk][:, vc * 512:vc * 512 + n],
                            start=(kk == 0), stop=(kk == DT - 1))
                    ot = wp.tile([128, 512], F32, tag="ot", bufs=6,
                                 name=f"ot{vb0}_{tt}_{vc}_{nm}")
                    nc.vector.tensor_copy(ot[:, :n], ps[:, :n])
                    nc.sync.dma_start(
                        self.out[tt * 128:(tt + 1) * 128,
                                 vb0 + vc * 512:vb0 + vc * 512 + n],
                        ot[:, :n])

    def _scales_1d(self, src, name):
        nc, wp = self.nc, self.wp
        t = wp.tile([128, DT], F32, tag="sc", bufs=14, name=name)
        nc.sync.dma_start(t[:], src.rearrange("(t p) -> p t", p=128))
        return t


def self_kg_slice(kg, r, h, h2):
    return kg[:, r, h, h2 * 128:(h2 + 1) * 128]


# ------------------------------------------------------------------ host side

_CACHE = {}


def _prep_inputs(inputs):
    ids = np.asarray(inputs["input_ids"])
    tok_emb = np.asarray(inputs["tok_emb"], dtype=np.float32)
    pos_emb = np.asarray(inputs["pos_emb"], dtype=np.float32)
    x = tok_emb[ids] + pos_emb[:L][None]          # [2, 1024, 768] f32

    bf = lambda a: np.ascontiguousarray(np.asarray(a, np.float32)).astype(ml_dtypes.bfloat16)
    f32 = lambda a: np.ascontiguousarray(np.asarray(a, np.float32))

    shared = {
        "wq": bf(inputs["wq"]), "wk": bf(inputs["wk"]),
        "wv": bf(inputs["wv"]), "wo": bf(inputs["wo"]),
        "w1": bf(inputs["w1_k"]), "w2": bf(inputs["w2_k"]),
        "w1b": f32(inputs["w1_b"]), "w2b": f32(inputs["w2_b"]),
        "ln1s": f32(inputs["ln1_s"]), "ln1b": f32(inputs["ln1_b"]),
        "ln2s": f32(inputs["ln2_s"]), "ln2b": f32(inputs["ln2_b"]),
        "lnfs": f32(inputs["lnf_s"]), "lnfb": f32(inputs["lnf_b"]),
    }
    head_bf = bf(inputs["head"])

    in_maps = []
    for c in range(N_CORES):
        b, j = c // 4, c % 4
        m = dict(shared)
        m["x0"] = np.ascontiguousarray(x[b, 256 * j:256 * (j + 1)].T)
        m["headw"] = np.ascontiguousarray(head_bf[:, c * VS:(c + 1) * VS])
        am = np.zeros((8, 128, TOK), np.float32)
        for kt in range(8):
            r, h2 = kt // 2, kt % 2
            kgl = 256 * r + 128 * h2 + np.arange(128)[:, None]
            qgl = 256 * j + np.arange(TOK)[None, :]
            am[kt] = np.where(kgl <= qgl, 0.0, NEG)
        m["amask"] = am
        in_maps.append(m)
    return in_maps


def _assemble(results):
    final = np.empty((B, L, VOC), np.float32)
    for c in range(N_CORES):
        o = results[c]["out"]                     # [2048, VS]
        for r in range(N_CORES):
            final[r // 4, 256 * (r % 4):256 * (r % 4 + 1),
                  c * VS:(c + 1) * VS] = o[256 * r:256 * (r + 1)]
    return final


def kernel(**inputs):
    if "k" not in _CACHE:
        _CACHE["k"] = GptKernel(reps=1)
    gk = _CACHE["k"]
    in_maps = _prep_inputs(inputs)
    res = bass_utils.run_bass_kernel_spmd(
        gk.nc, in_maps, core_ids=list(range(N_CORES)))
    _CACHE["last_results"] = res
    return _assemble(res.results)

